# revision 1
# baseline (speedup 1.0000x reference)
"""Trainium2 Bass kernel for nn_Net_21852793602541 (gnn_message_passing).

The reference net's output depends only on a tiny dependency cone of the
message-passing graph: the final hidden layer reads the wave-2 snapshot of
neuron activations, so only neurons feeding neuron 255 through channels whose
source was already processed matter.  For the fixed graph that is a 3-conv
chain (x -> n0 -> n172 -> n215), one 784->200 FC block, a 200->10 FC and
log_softmax.  The cone is recomputed at runtime from the src/tgt inputs.

Per-core mapping (data-parallel over batch, 16 images/core on 8 cores):
  * 5x5 conv == one PE accumulation group: contraction K = (dy, slot-row)
    (5 row-offsets x 32 padded columns = 160 -> matmuls of K=128/32) with a
    banded-Toeplitz stationary (fp16) against 5 y-shifted slot copies of the
    padded image block (fp16), N = (batch, y) = 448.  relu+bias fuses into
    the PSUM->SBUF stage; slot copies run on DVE in 16-bit 4x mode.
  * fc1 streams the 200 hidden units as the moving operand (7 accumulated
    matmuls, stationary = 16-wide activation slices), transposes the
    [16, 200] result back to hidden-on-partitions for bias+relu and fc2.
  * log_softmax: PE transpose, DVE max, ACT exp with fused accumulation,
    ACT ln (table preloaded at kernel start to keep it off the tail).
"""

import numpy as np

import concourse.bass as bass
import concourse.tile as tile
from concourse import bacc, mybir
from concourse.bass_utils import run_bass_kernel_spmd

# The axon NTFF profile hook normally lives in antenv.axon_hooks, which this
# image lacks.  Shim it from the boot module's ctypes implementation so
# BASS_TRACE=1 profiling works; degrade silently if unavailable.
try:
    import antenv.axon_hooks  # noqa: F401
except ImportError:
    try:
        import sys as _sys
        import types as _types

        from trn_agent_boot.trn_boot import _ntff_profile_via_ctypes

        _hook = _ntff_profile_via_ctypes('/opt/axon/libaxon_pjrt.so')
        _mod = _types.ModuleType('antenv.axon_hooks')
        _mod.get_axon_ntff_profile_hook = lambda: _hook
        _mod.set_axon_ntff_profile_hook = lambda h: None
        _sys.modules['antenv.axon_hooks'] = _mod
    except Exception:
        pass

F32 = mybir.dt.float32
F16 = mybir.dt.float16
AF = mybir.ActivationFunctionType
N_NEURONS = 256
N_CORES = 8
B_TOTAL = 128
B = B_TOTAL // N_CORES  # 16 images per core
HW = 28
FC_HID = 200
N_CLS = 10

LAST_RESULT = None  # BassKernelResults of the most recent run (for profiling)


# ---------------------------------------------------------------- schedule
def _schedule(src, tgt):
    n = N_NEURONS
    in_lists = [src[np.where(tgt == i)[0]].astype(np.int64).tolist() for i in range(n)]
    waves = []
    processed = np.zeros(n, bool)
    frontier = [0]
    while True:
        waves.append(list(frontier))
        processed[frontier] = True
        if processed[n - 1]:
            break
        nxt = set()
        for v in frontier:
            for m in tgt[src == v]:
                if not processed[m]:
                    nxt.add(int(m))
        frontier = sorted(nxt)
        assert frontier, "last neuron unreachable"
    return in_lists, waves


def _cone(src, tgt):
    """Returns (steps, fc_live).

    steps: ordered list of (node, [(srckey, channel), ...]) where srckey is
      'x' for the image input or an int neuron id computed in an earlier step.
    fc_live: [(channel_of_255, src_node), ...] live channels of the readout.
    """
    n = N_NEURONS
    in_lists, waves = _schedule(src, tgt)
    wave_of = {}
    for wi, w in enumerate(waves):
        for v in w:
            if v not in wave_of:
                wave_of[v] = wi
    BIG = 1 << 30
    w255 = wave_of[n - 1]
    fc_live = [(c, int(s)) for c, s in enumerate(in_lists[n - 1])
               if wave_of.get(int(s), BIG) < w255]

    live = {}
    stack = [s for _, s in fc_live]
    seen = set()
    while stack:
        v = stack.pop()
        if v in seen:
            continue
        seen.add(v)
        if v == 0:
            live[0] = [('x', 0)]
            continue
        chans = [(int(s), c) for c, s in enumerate(in_lists[v])
                 if wave_of.get(int(s), BIG) < wave_of[v]]
        assert chans, f"cone node {v} has no live channels"
        live[v] = [(s, c) for s, c in chans]
        stack += [s for s, _ in chans]

    steps = sorted(live.items(), key=lambda kv: wave_of[kv[0]])
    return steps, fc_live


# ---------------------------------------------------------- host-side packing
def _toeplitz(w):
    """w [5,5] -> [160, 28] banded matrix over K=(dy, row).

    Slot row r of each 32-row group holds padded-image column (r+2) mod 32,
    so the activation value at x lands at row x (32-aligned writes; wrapped
    rows 28..31 hold the zero x-padding)."""
    T = np.zeros((160, HW), np.float32)
    for dy in range(5):
        for dx in range(5):
            for xc in range(HW):
                T[dy * 32 + (xc + dx - 2) % 32, xc] = w[dy, dx]
    return T


def _xstack(xb):
    """xb [B,28,28] -> [160, B*32] fp16: five y-shifted padded slot copies.

    Slot_dy[r, b*32+yp] = xpad[b, yp+dy-2, (r+2) % 32]."""
    xpad = np.zeros((B, 32, 32), np.float32)
    xpad[:, 2:30, 2:30] = xb
    st = np.zeros((5, 32, B, 32), np.float32)
    for dy in range(5):
        lo, hi = max(0, 2 - dy), min(32, 34 - dy)
        st[dy, :, :, lo:hi] = xpad[:, lo + dy - 2:hi + dy - 2, :].transpose(2, 0, 1)
    st = np.roll(st, -2, axis=1)
    return st.reshape(160, B * 32).astype(np.float16)


class _Layout:
    def __init__(self):
        self.n = 0

    def alloc(self, w):
        c0 = self.n
        self.n += w
        return c0


def _pack(steps, fc_live, conv_w, conv_b, fc1_w, fc1_b, fc2_w, fc2_b):
    """Builds consts (f32), mainh-toeplitz block (fp16), f1w (fp16)."""
    slots = {}
    lay32 = _Layout()
    lay16 = _Layout()
    for v, chans in steps:
        for j, _ in enumerate(chans):
            slots[('toepA', v, j)] = lay16.alloc(HW)
            slots[('toepB', v, j)] = lay16.alloc(HW)
        slots[('cb', v)] = lay32.alloc(1)
    slots['fc1bA'] = lay32.alloc(1)
    slots['fc1bB'] = lay32.alloc(1)
    slots['fc2b'] = lay32.alloc(1)
    slots['ident'] = lay32.alloc(B)
    slots['fc2wA'] = lay32.alloc(N_CLS)
    slots['fc2wB'] = lay32.alloc(N_CLS)
    toep_cols = lay16.n
    slots['xsA'] = lay16.alloc(512)
    slots['xsB'] = lay16.alloc(512)

    C = np.zeros((128, lay32.n), np.float32)
    TH = np.zeros((128, toep_cols), np.float16)
    for v, chans in steps:
        for j, (skey, ch) in enumerate(chans):
            T = _toeplitz(conv_w[v, 0, ch])
            TH[:, slots[('toepA', v, j)]:slots[('toepA', v, j)] + HW] = T[:128]
            TH[:32, slots[('toepB', v, j)]:slots[('toepB', v, j)] + HW] = T[128:]
        C[:HW, slots[('cb', v)]] = conv_b[v]
    C[:128, slots['fc1bA']] = fc1_b[:128]
    C[:FC_HID - 128, slots['fc1bB']] = fc1_b[128:]
    C[:N_CLS, slots['fc2b']] = fc2_b
    C[:B, slots['ident']:slots['ident'] + B] = np.eye(B, dtype=np.float32)
    w2t = fc2_w.T  # [200, 10]
    C[:, slots['fc2wA']:slots['fc2wA'] + N_CLS] = w2t[:128]
    C[:FC_HID - 128, slots['fc2wB']:slots['fc2wB'] + N_CLS] = w2t[128:]

    n_fc = len(fc_live)
    f1p = np.zeros((128, 1400 * n_fc), np.float16)
    for k, (c, s) in enumerate(fc_live):
        blk = fc1_w[:, c * 784:(c + 1) * 784].reshape(FC_HID, HW, HW)  # [h, y, x]
        arr = blk.reshape(FC_HID, 4, 7, HW).transpose(1, 3, 2, 0)  # [yg, x, ysub, h]
        f1p[:, k * 1400:(k + 1) * 1400] = np.pad(
            arr, ((0, 0), (0, 4), (0, 0), (0, 0))).reshape(128, 1400)
    return C, TH, f1p, slots


# ---------------------------------------------------------- device program
def _build(steps, fc_live, ncols32, ncols16, nfc):
    nc = bacc.Bacc("TRN2", target_bir_lowering=False)
    consts_d = nc.dram_tensor("consts", [128, ncols32], F32, kind="ExternalInput")
    mainh_d = nc.dram_tensor("mainh", [128, ncols16], F16, kind="ExternalInput")
    f1w_d = nc.dram_tensor("f1w", [128, 1400 * nfc], F16, kind="ExternalInput")
    out_d = nc.dram_tensor("out", [B, N_CLS], F32, kind="ExternalOutput")

    feeds_conv = set()
    for v, chans in steps:
        for skey, _ in chans:
            if skey != 'x':
                feeds_conv.add(skey)
    fc_srcs = [s for _, s in fc_live]
    SL = _SLOTS

    with tile.TileContext(nc) as tc:
        with (
            tc.tile_pool(name="persist", bufs=1) as pool,
            tc.tile_pool(name="tmp", bufs=3) as tpool,
            tc.tile_pool(name="cpsum", bufs=1, space="PSUM") as cpp,
            tc.tile_pool(name="fpsum", bufs=1, space="PSUM") as fpp,
        ):
            # input DMAs, issue cost spread across engine sequencers
            consts = pool.tile([128, ncols32], F32, tag="consts")
            mainh = pool.tile([128, ncols16], F16, tag="mainh")
            f1w = pool.tile([128, 1400 * nfc], F16, tag="f1w")
            t3 = ncols16 // 3
            nc.sync.dma_start(mainh[:, :t3], mainh_d[:, :t3])
            nc.sync.dma_start(mainh[:, t3:2 * t3], mainh_d[:, t3:2 * t3])
            nc.sync.dma_start(mainh[:, 2 * t3:], mainh_d[:, 2 * t3:])
            nc.scalar.dma_start(consts[:], consts_d[:])
            fh = (1400 * nfc) // 2
            nc.gpsimd.dma_start(f1w[:, :fh], f1w_d[:, :fh])
            nc.gpsimd.dma_start(f1w[:, fh:], f1w_d[:, fh:])

            # preload the Exp activation table off the critical tail (Ln's
            # load would only evict it again -- single-slot table)
            swu = pool.tile([1, 2], F32, tag="swu")
            nc.vector.memset(swu[:], 1.0)
            nc.scalar.activation(swu[:, 0:1], swu[:, 0:1], AF.Exp)

            # lean PE warm-up: ~3us of back-to-back fp16 matmuls during the
            # input-DMA wait trips the HAM clock gate to 2.4 GHz before conv1
            dmy = pool.tile([1, 512], F16, tag="dmy")
            nc.gpsimd.memset(dmy[:], 1.0)
            warmps = fpp.tile([1, 512], F32, tag="l2", bufs=1)
            for _ in range(7):
                nc.tensor.matmul(warmps[:], dmy[:1, 0:1], dmy[:],
                                 start=True, stop=True)


            # activation slot tiles per conv producer (fp16, zero borders)
            stacks = {}
            for v in sorted(feeds_conv):
                a = pool.tile([128, B * 32], F16, name=f"stA_{v}", tag=f"stA_{v}")
                b = pool.tile([32, B * 32], F16, name=f"stB_{v}", tag=f"stB_{v}")
                nc.vector.memset(a[:], 0.0)
                nc.vector.memset(b[:], 0.0)
                stacks[v] = (a, b)
            fcstacks = {}
            for sv in set(fc_srcs):
                t = pool.tile([128, 112], F16, name=f"fcst_{sv}", tag=f"fcst_{sv}")
                nc.scalar.memzero(t[:])
                fcstacks[sv] = t

            def slot_slices(key):
                if key == 'x':
                    av = mainh[:, SL['xsA']:SL['xsA'] + 512]
                    bv = mainh[0:32, SL['xsB']:SL['xsB'] + 512]
                else:
                    a, b = stacks[key]
                    av, bv = a[:], b[:]
                return (av.rearrange("p (b y) -> p b y", y=32),
                        bv.rearrange("p (b y) -> p b y", y=32))

            # --- conv chain ---
            for v, chans in steps:
                cb0 = SL[('cb', v)]
                bias = consts[:HW, cb0:cb0 + 1]
                nch = len(chans)
                fc_only = v in fcstacks and v not in feeds_conv

                if fc_only:
                    # split the output into y-quarters so the fcstack
                    # PSUM->SBUF writes pipeline with the matmuls
                    fst = fcstacks[v]
                    fv = fst[:].rearrange("p (b s) -> p b s", s=7)
                    for g in range(4):
                        psq = cpp.tile([HW, B * 7], F32, tag="convq", bufs=2,
                                       name=f"psq{v}_{g}")
                        for j, (skey, ch) in enumerate(chans):
                            a0 = SL[('toepA', v, j)]
                            b0 = SL[('toepB', v, j)]
                            av, bv = slot_slices(skey)
                            ysl = slice(2 + 7 * g, 2 + 7 * g + 7)
                            nc.tensor.matmul(psq[:], mainh[:, a0:a0 + HW],
                                             av[:, :, ysl],
                                             start=(j == 0), stop=False)
                            nc.tensor.matmul(psq[:], mainh[:32, b0:b0 + HW],
                                             bv[:, :, ysl],
                                             start=False, stop=(j == nch - 1))
                        dst = fv[g * 32:g * 32 + HW, :, :]
                        if g % 2 == 0:
                            nc.scalar.activation(dst, psq[:], AF.Relu,
                                                 bias=bias, scale=1.0)
                        else:
                            nc.vector.tensor_scalar(
                                dst, psq[:], bias, 0.0,
                                mybir.AluOpType.add, mybir.AluOpType.max)
                    continue

                ps = cpp.tile([HW, B * HW], F32, tag="convps")
                for j, (skey, ch) in enumerate(chans):
                    a0 = SL[('toepA', v, j)]
                    b0 = SL[('toepB', v, j)]
                    av, bv = slot_slices(skey)
                    nc.tensor.matmul(ps[:], mainh[:, a0:a0 + HW], av[:, :, 2:30],
                                     start=(j == 0), stop=False)
                    nc.tensor.matmul(ps[:], mainh[:32, b0:b0 + HW], bv[:, :, 2:30],
                                     start=False, stop=(j == nch - 1))
                psv = ps[:].rearrange("p (b y) -> p b y", y=HW)

                if v in feeds_conv:
                    # relu+bias lands directly in slot group dy=2 (the
                    # unshifted copy); DVE replicates it to the other slots
                    av, bv = slot_slices(v)
                    g2 = av[64:64 + HW, :, 2:30]
                    nc.scalar.activation(g2, ps[:], AF.Relu, bias=bias, scale=1.0)
                    for dy in (0, 1, 3):
                        nc.vector.tensor_copy(
                            av[dy * 32:dy * 32 + HW, :, 4 - dy:32 - dy], g2)
                    nc.vector.tensor_copy(bv[0:HW, :, 0:28], g2)
                if v in fcstacks and not fc_only:
                    fst = fcstacks[v]
                    fv = fst[:].rearrange("p (b s) -> p b s", s=7)
                    for g in range(4):
                        dst = fv[g * 32:g * 32 + HW, :, :]
                        srcp = psv[:, :, 7 * g:7 * g + 7]
                        if g % 2 == 0:
                            nc.scalar.activation(dst, srcp, AF.Relu,
                                                 bias=bias, scale=1.0)
                        else:
                            nc.vector.tensor_scalar(
                                dst, srcp, bias, 0.0,
                                mybir.AluOpType.add, mybir.AluOpType.max)

            # --- fc1: activations stationary, hidden units streamed ---
            p1 = fpp.tile([B, FC_HID], F32, tag="p1")
            for k in range(nfc):
                fst = fcstacks[fc_live[k][1]]
                fv = fst[:].rearrange("p (b s) -> p b s", s=7)
                for sj in range(7):
                    i = k * 7 + sj
                    nc.tensor.matmul(p1[:], fv[:, :, sj:sj + 1],
                                     f1w[:, (k * 7 + sj) * 200:(k * 7 + sj + 1) * 200],
                                     start=(i == 0), stop=(i == 7 * nfc - 1))
            ht = pool.tile([B, FC_HID], F32, tag="ht")
            nc.scalar.copy(ht[:], p1[:])
            idn = consts[:B, SL['ident']:SL['ident'] + B]
            t1 = fpp.tile([128, B], F32, tag="tt", bufs=2)
            t2 = fpp.tile([FC_HID - 128, B], F32, tag="tt", bufs=2)
            nc.tensor.transpose(t1[:], ht[:, 0:128], idn)
            nc.tensor.transpose(t2[:], ht[:, 128:FC_HID], idn)
            h1 = pool.tile([128, B], F32, tag="h1")
            h2 = pool.tile([FC_HID - 128, B], F32, tag="h2")
            nc.scalar.activation(h1[:], t1[:], AF.Relu,
                                 bias=consts[:128, SL['fc1bA']:SL['fc1bA'] + 1],
                                 scale=1.0)
            nc.scalar.activation(h2[:], t2[:], AF.Relu,
                                 bias=consts[:FC_HID - 128, SL['fc1bB']:SL['fc1bB'] + 1],
                                 scale=1.0)

            # --- fc2 + log_softmax ---
            ps2 = fpp.tile([N_CLS, B], F32, tag="l2", bufs=1)
            nc.tensor.matmul(ps2[:], consts[:, SL['fc2wA']:SL['fc2wA'] + N_CLS],
                             h1[:], start=True, stop=False)
            nc.tensor.matmul(ps2[:], consts[:FC_HID - 128, SL['fc2wB']:SL['fc2wB'] + N_CLS],
                             h2[:], start=False, stop=True)
            lsb = pool.tile([N_CLS, B], F32, tag="logits")
            nc.scalar.activation(lsb[:], ps2[:], AF.Identity,
                                 bias=consts[:N_CLS, SL['fc2b']:SL['fc2b'] + 1],
                                 scale=1.0)
            pst = fpp.tile([B, N_CLS], F32, tag="l2", bufs=1)
            nc.tensor.transpose(pst[:], lsb[:], idn[:N_CLS, :N_CLS])
            nmx = pool.tile([B, 1], F32, tag="nmx")
            nc.vector.reduce_max(nmx[:], pst[:], axis=mybir.AxisListType.X,
                                 negate=True)
            ex = pool.tile([B, N_CLS], F32, tag="ex")
            sm = pool.tile([B, 1], F32, tag="sm")
            nc.scalar.activation(ex[:], pst[:], AF.Exp, bias=nmx[:], scale=1.0,
                                 accum_out=sm[:])
            lse = pool.tile([B, 1], F32, tag="lse")
            nc.scalar.activation(lse[:], sm[:], AF.Ln, bias=0.0, scale=1.0)
            ntot = pool.tile([B, 1], F32, tag="ntot")
            nc.vector.tensor_sub(ntot[:], nmx[:], lse[:])
            res = pool.tile([B, N_CLS], F32, tag="res")
            nc.scalar.activation(res[:], pst[:], AF.Identity, bias=ntot[:], scale=1.0)
            nc.sync.dma_start(out_d[:], res[:])
    nc.compile()
    return nc


_SLOTS = None
_PROG_CACHE = {}


def kernel(x, src, tgt, conv_w, conv_b, fc1_w, fc1_b, fc2_w, fc2_b):
    global _SLOTS, LAST_RESULT
    x = np.asarray(x, np.float32)
    src = np.asarray(src, np.int32)
    tgt = np.asarray(tgt, np.int32)
    conv_w = np.asarray(conv_w, np.float32)
    conv_b = np.asarray(conv_b, np.float32)
    fc1_w = np.asarray(fc1_w, np.float32)
    fc1_b = np.asarray(fc1_b, np.float32)
    fc2_w = np.asarray(fc2_w, np.float32)
    fc2_b = np.asarray(fc2_b, np.float32)

    steps, fc_live = _cone(src, tgt)
    C, TH, f1p, slots = _pack(steps, fc_live, conv_w, conv_b,
                              fc1_w, fc1_b, fc2_w, fc2_b)
    _SLOTS = slots
    ncols16 = TH.shape[1] + 1024

    key = (tuple((v, tuple(ch)) for v, ch in steps), tuple(fc_live),
           C.shape[1], ncols16)
    if key not in _PROG_CACHE:
        _PROG_CACHE[key] = _build(steps, fc_live, C.shape[1], ncols16,
                                  len(fc_live))
    nc = _PROG_CACHE[key]

    xs = x[:, 0]  # [128, 28, 28]
    in_maps = []
    for c in range(N_CORES):
        st = _xstack(xs[c * B:(c + 1) * B])
        xsB = np.zeros((128, 512), np.float16)
        xsB[:32] = st[128:160]
        mainh = np.concatenate([TH, st[:128], xsB], axis=1)
        in_maps.append({"consts": C, "mainh": mainh, "f1w": f1p})

    LAST_RESULT = run_bass_kernel_spmd(nc, in_maps, list(range(N_CORES)))
    out = np.concatenate([r["out"] for r in LAST_RESULT.results], axis=0)
    return out.astype(np.float32)



# revision 4
# speedup vs baseline: 1.1346x; 1.1346x over previous
"""Trainium2 Bass kernel for nn_Net_21852793602541 (gnn_message_passing).

The reference net's output depends only on a tiny dependency cone of the
message-passing graph: the final hidden layer reads a snapshot of neuron
activations, so only neurons feeding neuron 255 through channels whose
source was already processed matter.  For the fixed graph that is a 3-conv
chain (x -> n0 -> n172 -> n215), one 784->200 FC block, a 200->10 FC and
log_softmax.  The cone is recomputed at runtime from the src/tgt inputs.

Per-core mapping (data-parallel over batch, 16 images/core on 8 cores):
  * each 5x5 conv = 5 accumulating K=32 matmuls: stationary T_dy [32,28]
    (x-banded weights), moving = ONE [32, B*32] activation tile read at 5
    different y-column offsets.  No slot replication between convs: the
    relu+bias PSUM->SBUF write (split ACT/DVE) is the only inter-conv op.
  * fc1 runs transposed: stationary = fc1-weight slices, moving = the
    (yg,x)-packed activation b-columns, accumulating hidden as [h, b]
    directly in PSUM; relu+bias is a per-partition ACT op, feeding fc2 as
    the stationary operand to produce logits in [b, cls] orientation with
    fc2_b folded in via a constant-1 contraction row.
  * log_softmax without max-subtraction (logits are O(1)); exp-with-
    accumulation, ln, and one DVE tensor_scalar subtract.
  * all activations (relu/copy/exp/ln) are steered to the single
    natural_log_exp_and_others table so exactly one ACT_TABLE_LOAD runs,
    during the input-DMA wait.
"""

import numpy as np

import concourse.bass as bass
import concourse.tile as tile
from concourse import bacc, mybir
from concourse.bass_utils import run_bass_kernel_spmd

# The axon NTFF profile hook normally lives in antenv.axon_hooks, which this
# image lacks.  Shim it from the boot module's ctypes implementation so
# BASS_TRACE=1 profiling works; degrade silently if unavailable.
try:
    import antenv.axon_hooks  # noqa: F401
except ImportError:
    try:
        import sys as _sys
        import types as _types

        from trn_agent_boot.trn_boot import _ntff_profile_via_ctypes

        _hook = _ntff_profile_via_ctypes('/opt/axon/libaxon_pjrt.so')
        _mod = _types.ModuleType('antenv.axon_hooks')
        _mod.get_axon_ntff_profile_hook = lambda: _hook
        _mod.set_axon_ntff_profile_hook = lambda h: None
        _sys.modules['antenv.axon_hooks'] = _mod
    except Exception:
        pass

# Steer every activation we use (exp/ln/relu/identity/copy) to the one
# act-func table that contains them all, so the compiler's table-load pass
# emits a single ACT_TABLE_LOAD (id semantics are preserved: the dict keeps
# the full act_info.json entry order, only the non-target sets are thinned).
import concourse.hw_specs as _hw_specs
import concourse.bacc as _bacc_mod

_orig_get_act_tables = _hw_specs.get_activation_tables


def _patched_act_tables(arch):
    tabs = _orig_get_act_tables(arch)
    target = 'natural_log_exp_and_others'
    if target not in tabs:
        return tabs
    t6 = tabs[target]
    return {name: (set(s) if name == target else set(s) - t6)
            for name, s in tabs.items()}


_hw_specs.get_activation_tables = _patched_act_tables
_bacc_mod.get_activation_tables = _patched_act_tables

F32 = mybir.dt.float32
F16 = mybir.dt.float16
AF = mybir.ActivationFunctionType
ALU = mybir.AluOpType
N_NEURONS = 256
N_CORES = 8
B_TOTAL = 128
B = B_TOTAL // N_CORES  # 16 images per core
HW = 28
FC_HID = 200
N_CLS = 10

LAST_RESULT = None  # BassKernelResults of the most recent run (for profiling)


# ---------------------------------------------------------------- schedule
def _schedule(src, tgt):
    n = N_NEURONS
    in_lists = [src[np.where(tgt == i)[0]].astype(np.int64).tolist() for i in range(n)]
    waves = []
    processed = np.zeros(n, bool)
    frontier = [0]
    while True:
        waves.append(list(frontier))
        processed[frontier] = True
        if processed[n - 1]:
            break
        nxt = set()
        for v in frontier:
            for m in tgt[src == v]:
                if not processed[m]:
                    nxt.add(int(m))
        frontier = sorted(nxt)
        assert frontier, "last neuron unreachable"
    return in_lists, waves


def _cone(src, tgt):
    """Returns (steps, fc_live).

    steps: ordered list of (node, [(srckey, channel), ...]) where srckey is
      'x' for the image input or an int neuron id computed in an earlier step.
    fc_live: [(channel_of_255, src_node), ...] live channels of the readout.
    """
    n = N_NEURONS
    in_lists, waves = _schedule(src, tgt)
    wave_of = {}
    for wi, w in enumerate(waves):
        for v in w:
            if v not in wave_of:
                wave_of[v] = wi
    BIG = 1 << 30
    w255 = wave_of[n - 1]
    fc_live = [(c, int(s)) for c, s in enumerate(in_lists[n - 1])
               if wave_of.get(int(s), BIG) < w255]

    live = {}
    stack = [s for _, s in fc_live]
    seen = set()
    while stack:
        v = stack.pop()
        if v in seen:
            continue
        seen.add(v)
        if v == 0:
            live[0] = [('x', 0)]
            continue
        chans = [(int(s), c) for c, s in enumerate(in_lists[v])
                 if wave_of.get(int(s), BIG) < wave_of[v]]
        assert chans, f"cone node {v} has no live channels"
        live[v] = [(s, c) for s, c in chans]
        stack += [s for s, _ in chans]

    steps = sorted(live.items(), key=lambda kv: wave_of[kv[0]])
    return steps, fc_live


# ---------------------------------------------------------- host-side packing
def _tband(w, dy):
    """w [5,5], one dy -> [32, 28] x-banded matrix: T[r, m] = w[dy, r-m+2].

    Contraction rows r are input x (image W columns); rows 28..31 multiply
    the zeroed x-padding rows of the activation tile."""
    T = np.zeros((32, HW), np.float32)
    for m in range(HW):
        for dx in range(5):
            r = m + dx - 2
            if 0 <= r < 32:
                T[r, m] = w[dy, dx]
    return T


def _xbase(xb):
    """xb [B,28,28] -> [32, B*32] fp16: X[j, b*32 + i+2] = xb[b, i, j]."""
    X = np.zeros((32, B, 32), np.float32)
    X[:HW, :, 2:30] = xb.transpose(2, 0, 1)
    return X.reshape(32, B * 32).astype(np.float16)


class _Layout:
    def __init__(self):
        self.n = 0

    def alloc(self, w):
        c0 = self.n
        self.n += w
        return c0


def _pack(steps, fc_live, conv_w, conv_b, fc1_w, fc1_b, fc2_w, fc2_b):
    """Builds consts (f32 [128, n32]), mainh (fp16 [32, n16]), f1w (fp16)."""
    slots = {}
    lay32 = _Layout()
    lay16 = _Layout()
    for v, chans in steps:
        for j, _ in enumerate(chans):
            for dy in range(5):
                slots[('t', v, j, dy)] = lay16.alloc(HW)
        slots[('cb', v)] = lay32.alloc(1)
    slots['fc1bA'] = lay32.alloc(1)
    slots['fc1bB'] = lay32.alloc(1)
    slots['fc2wA'] = lay32.alloc(N_CLS)
    slots['fc2wB'] = lay32.alloc(N_CLS)
    slots['xs'] = lay16.alloc(B * 32)

    C = np.zeros((128, lay32.n), np.float32)
    TH = np.zeros((32, lay16.n), np.float16)
    for v, chans in steps:
        for j, (skey, ch) in enumerate(chans):
            for dy in range(5):
                c0 = slots[('t', v, j, dy)]
                TH[:, c0:c0 + HW] = _tband(conv_w[v, 0, ch], dy)
        C[:HW, slots[('cb', v)]] = conv_b[v]
    C[:128, slots['fc1bA']] = fc1_b[:128]
    C[:FC_HID - 128, slots['fc1bB']] = fc1_b[128:]
    w2t = fc2_w.T  # [200, 10]
    C[:, slots['fc2wA']:slots['fc2wA'] + N_CLS] = w2t[:128]
    C[:FC_HID - 128, slots['fc2wB']:slots['fc2wB'] + N_CLS] = w2t[128:]
    C[FC_HID - 128, slots['fc2wB']:slots['fc2wB'] + N_CLS] = fc2_b  # 1s-row

    n_fc = len(fc_live)
    f1p = np.zeros((128, 1400 * n_fc), np.float16)
    for k, (c, s) in enumerate(fc_live):
        blk = fc1_w[:, c * 784:(c + 1) * 784].reshape(FC_HID, HW, HW)  # [h, i, j]
        arr = blk.reshape(FC_HID, 4, 7, HW).transpose(1, 3, 2, 0)  # [yg, j, ysub, h]
        f1p[:, k * 1400:(k + 1) * 1400] = np.pad(
            arr, ((0, 0), (0, 4), (0, 0), (0, 0))).reshape(128, 1400)
    return C, TH, f1p, slots


# ---------------------------------------------------------- device program
def _build(steps, fc_live, ncols32, ncols16, nfc):
    nc = bacc.Bacc("TRN2", target_bir_lowering=False)
    consts_d = nc.dram_tensor("consts", [128, ncols32], F32, kind="ExternalInput")
    mainh_d = nc.dram_tensor("mainh", [32, ncols16], F16, kind="ExternalInput")
    f1w_d = nc.dram_tensor("f1w", [128, 1400 * nfc], F16, kind="ExternalInput")
    out_d = nc.dram_tensor("out", [B, N_CLS], F32, kind="ExternalOutput")

    feeds_conv = set()
    for v, chans in steps:
        for skey, _ in chans:
            if skey != 'x':
                feeds_conv.add(skey)
    fc_srcs = [s for _, s in fc_live]
    SL = _SLOTS
    HB = FC_HID - 128  # 72

    with tile.TileContext(nc) as tc:
        with (
            tc.tile_pool(name="persist", bufs=1) as pool,
            tc.tile_pool(name="cpsum", bufs=2, space="PSUM") as cpp,
            tc.tile_pool(name="fpsum", bufs=1, space="PSUM") as fpp,
        ):
            consts = pool.tile([128, ncols32], F32, tag="consts")
            mainh = pool.tile([32, ncols16], F16, tag="mainh")
            f1w = pool.tile([128, 1400 * nfc], F16, tag="f1w")

            # warm-up feedstock + activation tiles, zeroed before the DMAs
            dmy = pool.tile([128, 512], F16, tag="dmy")
            nc.vector.memset(dmy[:], 1.0)
            acts = {}
            for v in sorted(feeds_conv):
                a = pool.tile([32, B * 32], F16, name=f"act_{v}", tag=f"act_{v}")
                nc.vector.memset(a[:], 0.0)
                acts[v] = a
            fcstacks = {}
            for sv in set(fc_srcs):
                t = pool.tile([128, 112], F16, name=f"fcst_{sv}", tag=f"fcst_{sv}")
                nc.vector.memset(t[:], 0.0)
                fcstacks[sv] = t
            h12 = pool.tile([128, 32], F32, tag="h12")
            # fc2 bias row: row HB=72 must read 1.0; memset a 32-aligned
            # block (64:80) — rows 64..71 are overwritten by the fc1-B relu,
            # rows 73..79 are never read (fc2 contraction stops at row 72)
            nc.vector.memset(h12[64:80, 16:32], 1.0)

            # single act-table load, hoisted into the DMA wait
            swu = pool.tile([1, 2], F32, tag="swu")
            nc.gpsimd.memset(swu[:], 1.0)
            nc.scalar.activation(swu[:, 0:1], swu[:, 0:1], AF.Exp)

            # input DMAs: critical tensors first, HWDGE rings only
            nc.sync.dma_start(mainh[:], mainh_d[:])
            nc.scalar.dma_start(consts[:], consts_d[:])
            fh = (1400 * nfc) // 2
            nc.sync.dma_start(f1w[:, :fh], f1w_d[:, :fh])
            nc.scalar.dma_start(f1w[:, fh:], f1w_d[:, fh:])

            # PE warm-up: full-K matmuls during the input-DMA wait to trip
            # the HAM clock gate to 2.4 GHz before the conv chain
            warmps = fpp.tile([128, 512], F32, tag="warm", bufs=1)
            for _ in range(7):
                nc.tensor.matmul(warmps[:], dmy[:, 0:128], dmy[:],
                                 start=True, stop=True)

            xs0 = SL['xs']

            def src_view(skey):
                t = mainh[:, xs0:xs0 + B * 32] if skey == 'x' else acts[skey][:]
                return t.rearrange("p (b q) -> p b q", q=32)

            # --- conv chain: 5 accumulating K=32 matmuls per channel ---
            for v, chans in steps:
                cb0 = SL[('cb', v)]
                bias = consts[:HW, cb0:cb0 + 1]
                nch = len(chans)
                ps = cpp.tile([HW, B * HW], F32, tag="convps", name=f"ps{v}")
                mm = 0
                for j, (skey, ch) in enumerate(chans):
                    av = src_view(skey)
                    for dy in range(5):
                        t0 = SL[('t', v, j, dy)]
                        nc.tensor.matmul(ps[:], mainh[:, t0:t0 + HW],
                                         av[:, :, dy:dy + HW],
                                         start=(mm == 0), stop=(mm == 5 * nch - 1))
                        mm += 1
                psv = ps[:].rearrange("p (b y) -> p b y", y=HW)

                if v in acts:
                    # relu+bias straight into the act tile, split ACT/DVE
                    av = acts[v][:].rearrange("p (b q) -> p b q", q=32)
                    nc.scalar.activation(av[0:HW, :, 2:16], psv[:, :, 0:14],
                                         AF.Relu, bias=bias, scale=1.0)
                    nc.vector.tensor_scalar(av[0:HW, :, 16:30], psv[:, :, 14:28],
                                            bias, 0.0, ALU.add, ALU.max)
                if v in fcstacks:
                    fst = fcstacks[v]
                    fv = fst[:].rearrange("p (b s) -> p b s", s=7)
                    for g in range(4):
                        dst = fv[g * 32:g * 32 + HW, :, :]
                        srcp = psv[:, :, 7 * g:7 * g + 7]
                        if g % 2 == 0:
                            nc.scalar.activation(dst, srcp, AF.Relu,
                                                 bias=bias, scale=1.0)
                        else:
                            nc.vector.tensor_scalar(dst, srcp, bias, 0.0,
                                                    ALU.add, ALU.max)

            # --- fc1 transposed: hidden accumulates as [h, b] in PSUM ---
            p1a = fpp.tile([128, B], F32, tag="p1a")
            p1b = fpp.tile([HB, B], F32, tag="p1b")
            for k in range(nfc):
                fst = fcstacks[fc_live[k][1]]
                fv = fst[:].rearrange("p (b s) -> p b s", s=7)
                for sj in range(7):
                    i = k * 7 + sj
                    w0 = i * 200
                    nc.tensor.matmul(p1a[:], f1w[:, w0:w0 + 128], fv[:, :, sj],
                                     start=(i == 0), stop=(i == 7 * nfc - 1))
                    nc.tensor.matmul(p1b[:], f1w[:, w0 + 128:w0 + 200], fv[:, :, sj],
                                     start=(i == 0), stop=(i == 7 * nfc - 1))
            nc.scalar.activation(h12[:, 0:B], p1a[:], AF.Relu,
                                 bias=consts[:128, SL['fc1bA']:SL['fc1bA'] + 1],
                                 scale=1.0)
            nc.vector.tensor_scalar(h12[0:HB, 16:16 + B], p1b[:],
                                    consts[:HB, SL['fc1bB']:SL['fc1bB'] + 1],
                                    0.0, ALU.add, ALU.max)

            # --- fc2 into [b, cls] orientation, bias via the 1s-row ---
            ps2 = fpp.tile([B, N_CLS], F32, tag="l2")
            nc.tensor.matmul(ps2[:], h12[:, 0:B],
                             consts[:, SL['fc2wA']:SL['fc2wA'] + N_CLS],
                             start=True, stop=False)
            nc.tensor.matmul(ps2[:], h12[0:HB + 1, 16:16 + B],
                             consts[0:HB + 1, SL['fc2wB']:SL['fc2wB'] + N_CLS],
                             start=False, stop=True)

            # --- log_softmax: x - ln(sum(exp(x))), no max-subtraction ---
            ex = pool.tile([B, N_CLS], F32, tag="ex")
            sm = pool.tile([B, 1], F32, tag="sm")
            nc.scalar.activation(ex[:], ps2[:], AF.Exp, bias=0.0, scale=1.0,
                                 accum_out=sm[:])
            lse = pool.tile([B, 1], F32, tag="lse")
            nc.scalar.activation(lse[:], sm[:], AF.Ln, bias=0.0, scale=1.0)
            res = pool.tile([B, N_CLS], F32, tag="res")
            nc.vector.tensor_scalar_sub(res[:], ps2[:], lse[:])
            nc.sync.dma_start(out_d[:], res[:])
    nc.compile()
    return nc


_SLOTS = None
_PROG_CACHE = {}


def kernel(x, src, tgt, conv_w, conv_b, fc1_w, fc1_b, fc2_w, fc2_b):
    global _SLOTS, LAST_RESULT
    x = np.asarray(x, np.float32)
    src = np.asarray(src, np.int32)
    tgt = np.asarray(tgt, np.int32)
    conv_w = np.asarray(conv_w, np.float32)
    conv_b = np.asarray(conv_b, np.float32)
    fc1_w = np.asarray(fc1_w, np.float32)
    fc1_b = np.asarray(fc1_b, np.float32)
    fc2_w = np.asarray(fc2_w, np.float32)
    fc2_b = np.asarray(fc2_b, np.float32)

    steps, fc_live = _cone(src, tgt)
    C, TH, f1p, slots = _pack(steps, fc_live, conv_w, conv_b,
                              fc1_w, fc1_b, fc2_w, fc2_b)
    _SLOTS = slots
    ncols16 = TH.shape[1] + B * 32

    key = (tuple((v, tuple(ch)) for v, ch in steps), tuple(fc_live),
           C.shape[1], ncols16)
    if key not in _PROG_CACHE:
        _PROG_CACHE[key] = _build(steps, fc_live, C.shape[1], ncols16,
                                  len(fc_live))
    nc = _PROG_CACHE[key]

    xs = x[:, 0]  # [128, 28, 28]
    in_maps = []
    for c in range(N_CORES):
        mainh = np.concatenate([TH, _xbase(xs[c * B:(c + 1) * B])], axis=1)
        in_maps.append({"consts": C, "mainh": mainh, "f1w": f1p})

    LAST_RESULT = run_bass_kernel_spmd(nc, in_maps, list(range(N_CORES)))
    out = np.concatenate([r["out"] for r in LAST_RESULT.results], axis=0)
    return out.astype(np.float32)
